# revision 1
# baseline (speedup 1.0000x reference)
"""MultiHeadAttention (faithful raw-reshape variant) on 8 trn2 NeuronCores.

Math (per batch b):
  Y  = Xq @ Wq.T            [S, D]
  Z  = Xk @ Wk.T            [S, D]
  V  = Xv @ Wv.T            [S, D]
  reshape (B,S,D)->(B,H,S,dk) is a *raw view*: head h <- rows [128h, 128h+128)
  of Y/Z/V; within the block, q = 16t + j maps to (row t, features 64j..64j+64).
  A  = softmax(Qh @ Kh.T / 8), O = A @ Vh, placed back into the same raw view,
  out = Hcat @ Wo.T + b_o.

Because heads partition the *rows* of Y/Z/V, the computation is fully
independent across (b, h): 32 tasks, 4 per core, no collectives.

Per-core device program (heads hl=0..3 over the core's 512 rows):
  QT/KT: transposed projections  QT[f, s] (f on partitions, 8x128 chunks)
  V:     normal orientation with a 32-wide ones block -> [128, 16, 96]
  scores (transposed): AT[t', q] = sum_k KT[64j'+k, t'] QT[64j+k, t];
         query blocks of opposite j-parity run as pairs on PE row groups
         0-63 / 64-127 (row tiling -> concurrent, 2x score throughput)
  exp on ACT with scale=1/8 fused; no max subtraction (scores ~N(0,1), fp32
         exp cannot overflow); one ACTIVATE per PSUM bank (2-bank reads hang)
  AV:    O.T[k', q] += V[:, j', :].T @ expAT ; rows 64-95 = denominator copies
  normalize: DVE reciprocal + 32-wide quadrant-aligned multiplies into HcatT
  out:   out[t, f'] = HcatT.T @ WoT + b_o

Fallback: TRN_MM_DTYPE=f32 env selects exact fp32 matmuls (~1.5x slower).
"""

import os

import numpy as np

import concourse.bass as bass
import concourse.mybir as mybir
import concourse.tile as tile
from concourse import bacc

B, S, D = 2, 2048, 1024
H, DK = 16, 64
NCORES = 8
HPC = H // (NCORES // B)  # heads per core = 4
SC = HPC * 128            # s-rows per core = 512
P = 128
KD = D // P               # 8 contraction chunks
PO = D // P               # 8 feature chunks
F32 = mybir.dt.float32

# matmul dtype mode: "f32" | "f32r" | "bf16"
MODE = os.environ.get("TRN_MM_DTYPE", "bf16")


def _mm_dt():
    return {"f32": mybir.dt.float32, "f32r": mybir.dt.float32,
            "bf16": mybir.dt.bfloat16}[MODE]


def _np_dt():
    import ml_dtypes
    return {"f32": np.float32, "f32r": np.float32,
            "bf16": ml_dtypes.bfloat16}[MODE]


def _c(ap):
    """Bitcast matmul operands to float32r in f32r mode."""
    if MODE == "f32r":
        return ap.bitcast(mybir.dt.float32r)
    return ap


def build_body(nc, out_ap, ins):
    """Emit the per-core program. ins: dict of DRAM APs."""
    xqt, xkt, xvt = ins["xqt"], ins["xkt"], ins["xvt"]
    wqt, wkt, wvt, wot = ins["wqt"], ins["wkt"], ins["wvt"], ins["wot"]
    bo = ins["bo"]
    mdt = _mm_dt()
    EXPF = mybir.ActivationFunctionType.Exp
    MULT = mybir.AluOpType.mult
    ADD = mybir.AluOpType.add

    wide = MODE == "bf16"  # fp32 storage doesn't fit double-buffered weights
    with tile.TileContext(nc) as tc:
        with (
            tc.tile_pool(name="singles", bufs=1) as singles,
            tc.tile_pool(name="wp", bufs=2 if wide else 1) as wp,
            tc.tile_pool(name="xp", bufs=2) as xp,
            tc.tile_pool(name="exp", bufs=10 if wide else 4) as exp_pool,
            tc.tile_pool(name="smalls", bufs=3) as smalls,
            tc.tile_pool(name="outp", bufs=4) as outp,
            tc.tile_pool(name="ps_mm", bufs=2, space="PSUM") as ps_mm,
            tc.tile_pool(name="ps_at", bufs=4, space="PSUM") as ps_at,
            tc.tile_pool(name="ps_o", bufs=2, space="PSUM") as ps_o,
        ):
            # --- constants ---
            bo_sb = singles.tile([P, D], F32, tag="bo", name="bo_sb")
            bo_bcast = bass.AP(tensor=bo.tensor, offset=bo.offset,
                               ap=[[0, P], list(bo.ap[-1])])
            nc.gpsimd.dma_start(out=bo_sb, in_=bo_bcast)

            qt_sb = singles.tile([P, PO, SC], mdt, tag="qt", name="qt_sb")
            kt_sb = singles.tile([P, PO, SC], mdt, tag="kt", name="kt_sb")
            # partition-rotated copy: kt2[pi] = kt[(pi+64) % 128], so a key
            # slice of either j'-parity is available at either partition base
            kt2_sb = singles.tile([P, PO, SC], mdt, tag="kt2", name="kt2_sb")
            hcat = singles.tile([P, PO, SC], mdt, tag="hcat", name="hcat")
            # [V | ones*32]: the A@V matmul then emits 32 copies of the
            # softmax denominator on partitions 64..95 (time is free: matmul
            # cost depends only on the moving-operand free size)
            v_sb = [singles.tile([P, 16, 96], mdt, tag=f"v{hl}", name=f"v_sb{hl}")
                    for hl in range(HPC)]
            for hl in range(HPC):
                nc.vector.memset(v_sb[hl][:, :, 64:96], 1.0)

            # --- projections ---
            # loads split by kd quarters: region-level deps let the first
            # projection matmuls start after 1/4 of the tensor lands
            def load_w(ap):
                t = wp.tile([P, KD, D], mdt, tag="w", name="w_t")
                src = ap.rearrange("(kd p) f -> p kd f", p=P)
                for q in range(0, KD, 2):
                    nc.sync.dma_start(t[:, q:q + 2], src[:, q:q + 2])
                return t

            def load_x(ap):
                t = xp.tile([P, KD, SC], mdt, tag="x", name="x_t")
                src = ap.rearrange("(kd p) s -> p kd s", p=P)
                for q in range(0, KD, 2):
                    nc.sync.dma_start(t[:, q:q + 2], src[:, q:q + 2])
                return t

            # Q/K transposed: QT[f, s] = sum_d WqT[d, f] XqT[d, s]
            for w_ap, x_ap, dst in ((wqt, xqt, qt_sb), (wkt, xkt, kt_sb)):
                w_t, x_t = load_w(w_ap), load_x(x_ap)
                for mf in range(PO):
                    ps = ps_mm.tile([P, SC], F32, tag="mm", name="ps")
                    for kd in range(KD):
                        nc.tensor.matmul(
                            ps, _c(w_t[:, kd, mf * P:(mf + 1) * P]),
                            _c(x_t[:, kd, :]),
                            start=(kd == 0), stop=(kd == KD - 1))
                    nc.vector.tensor_copy(dst[:, mf, :], ps)
                    if dst is kt_sb:
                        nc.sync.dma_start(kt2_sb[0:64, mf],
                                          kt_sb[64:128, mf])
                        nc.sync.dma_start(kt2_sb[64:128, mf],
                                          kt_sb[0:64, mf])

            # V normal: V[s, f] = sum_d XvT[d, s] WvT[d, f].
            # Only head 0's V up front; heads 1-3 are emitted after head 0's
            # first attention half so PE feeds ACT scores sooner (emission
            # order is dependency order in Tile, so v_proj(h) must still
            # precede head h's first A@V matmul).
            wv_t, xv_t = load_w(wvt), load_x(xvt)

            def v_proj(hl):
                for nf in range(2):
                    ps = ps_mm.tile([P, SC], F32, tag="mm", name="ps")
                    for kd in range(KD):
                        nc.tensor.matmul(
                            ps, _c(xv_t[:, kd, hl * P:(hl + 1) * P]),
                            _c(wv_t[:, kd, nf * 512:(nf + 1) * 512]),
                            start=(kd == 0), stop=(kd == KD - 1))
                    nc.vector.tensor_copy(
                        v_sb[hl][:, nf * 8:(nf + 1) * 8, 0:64],
                        ps.rearrange("p (j k) -> p j k", k=64))

            v_proj(0)

            wo_t = load_w(wot)  # prefetched during attention

            # --- attention per local head ---
            # Query blocks of opposite j-parity are processed in pairs: their
            # score matmuls run on PE row-groups 0-63 / 64-127 and execute
            # concurrently (row tiling), doubling score throughput.
            # outproj(h) is emitted after head h+1's first half so the next
            # head's scores reach PE at the boundary and ACT never starves.
            pending = []
            for hl in range(HPC):
                hs = slice(hl * P, (hl + 1) * P)
                for pp in range(2):  # po-half; qbA has a=0, qbB a=1
                    rhs_a = qt_sb[0:64, 4 * pp:4 * pp + 4, hs]
                    rhs_b = qt_sb[64:128, 4 * pp:4 * pp + 4, hs]
                    o_a = ps_o.tile([96, 512], F32, tag="o", name="o_a")
                    o_b = ps_o.tile([96, 512], F32, tag="o", name="o_b")
                    for jp in range(16):
                        a2, po2 = jp % 2, jp // 2
                        ksrc_a = kt_sb if a2 == 0 else kt2_sb
                        ksrc_b = kt_sb if a2 == 1 else kt2_sb
                        at_a = ps_at.tile([P, 512], F32, tag="at", name="at_a")
                        at_b = ps_at.tile([P, 512], F32, tag="at", name="at_b")
                        # adjacent matmuls on PE row groups 0-63 / 64-127
                        # execute concurrently (row tiling)
                        nc.tensor.matmul(at_a, _c(ksrc_a[0:64, po2, hs]),
                                         _c(rhs_a), start=True, stop=True)
                        nc.tensor.matmul(at_b, _c(ksrc_b[64:128, po2, hs]),
                                         _c(rhs_b), start=True, stop=True)
                        ex_a = exp_pool.tile([P, 512], mdt, tag="ex",
                                             name="ex_a")
                        ex_b = exp_pool.tile([P, 512], mdt, tag="ex",
                                             name="ex_b")
                        # note: a single ACTIVATE must not read >1 PSUM bank
                        # (2-bank reads hang the device)
                        nc.scalar.activation(ex_a, at_a, EXPF, scale=0.125)
                        nc.scalar.activation(ex_b, at_b, EXPF, scale=0.125)
                        st, sp = jp == 0, jp == 15
                        nc.tensor.matmul(o_a, _c(v_sb[hl][:, jp, :]),
                                         _c(ex_a), start=st, stop=sp)
                        nc.tensor.matmul(o_b, _c(v_sb[hl][:, jp, :]),
                                         _c(ex_b), start=st, stop=sp)
                    # normalize into HcatT: recip of the replicated denom
                    # rows, then 32-wide multiplies (quadrant-aligned)
                    for a, o_ps in ((0, o_a), (1, o_b)):
                        rc = smalls.tile([P, 512], F32, tag="rc", name="rc")
                        nc.vector.reciprocal(rc[64:96, :], o_ps[64:96, :])
                        dst = hcat[64 * a:64 * a + 64, 4 * pp:4 * pp + 4, hs]
                        for u in range(2):
                            nc.vector.tensor_tensor(
                                dst[32 * u:32 * u + 32],
                                o_ps[32 * u:32 * u + 32, :].rearrange(
                                    "k (c t) -> k c t", t=P),
                                rc[64:96, :].rearrange("k (c t) -> k c t", t=P),
                                MULT)

                    if pp == 0:
                        if hl == 0:
                            for h2 in range(1, HPC):
                                v_proj(h2)
                        for emit in pending:
                            emit()
                        pending = []

                # output projection for this head block (deferred emission)
                def outproj(hs=hs):
                    for nf in range(2):
                        fs = slice(nf * 512, (nf + 1) * 512)
                        ps = ps_mm.tile([P, 512], F32, tag="mm", name="ps")
                        for po in range(PO):
                            nc.tensor.matmul(
                                ps, _c(hcat[:, po, hs]), _c(wo_t[:, po, fs]),
                                start=(po == 0), stop=(po == PO - 1))
                        os_t = outp.tile([P, 512], F32, tag="os", name="os_t")
                        nc.vector.tensor_tensor(os_t, ps, bo_sb[:, fs], ADD)
                        nc.sync.dma_start(out_ap[hs, fs], os_t)
                pending.append(outproj)

            for emit in pending:
                emit()
    return nc


def build_program():
    nc = bacc.Bacc("TRN2", target_bir_lowering=False, debug=False,
                   enable_asserts=False, num_devices=NCORES)
    mdt = _mm_dt()
    ins = {
        "xqt": nc.dram_tensor("xqt", [D, SC], mdt, kind="ExternalInput").ap(),
        "xkt": nc.dram_tensor("xkt", [D, SC], mdt, kind="ExternalInput").ap(),
        "xvt": nc.dram_tensor("xvt", [D, SC], mdt, kind="ExternalInput").ap(),
        "wqt": nc.dram_tensor("wqt", [D, D], mdt, kind="ExternalInput").ap(),
        "wkt": nc.dram_tensor("wkt", [D, D], mdt, kind="ExternalInput").ap(),
        "wvt": nc.dram_tensor("wvt", [D, D], mdt, kind="ExternalInput").ap(),
        "wot": nc.dram_tensor("wot", [D, D], mdt, kind="ExternalInput").ap(),
        "bo": nc.dram_tensor("bo", [1, D], F32, kind="ExternalInput").ap(),
    }
    out_ap = nc.dram_tensor("out", [SC, D], F32, kind="ExternalOutput").ap()
    build_body(nc, out_ap, ins)
    nc.finalize()
    return nc


def make_in_maps(inputs):
    ndt = _np_dt()
    Xq = np.asarray(inputs["X_q"], dtype=np.float32)
    Xk = np.asarray(inputs["X_k"], dtype=np.float32)
    Xv = np.asarray(inputs["X_v"], dtype=np.float32)
    wqt = np.ascontiguousarray(np.asarray(inputs["W_q"], np.float32).T).astype(ndt)
    wkt = np.ascontiguousarray(np.asarray(inputs["W_k"], np.float32).T).astype(ndt)
    wvt = np.ascontiguousarray(np.asarray(inputs["W_v"], np.float32).T).astype(ndt)
    wot = np.ascontiguousarray(np.asarray(inputs["W_o"], np.float32).T).astype(ndt)
    bo = np.asarray(inputs["b_o"], np.float32).reshape(1, D)
    xt = {n: [np.ascontiguousarray(x[b].T).astype(ndt) for b in range(B)]
          for n, x in (("xqt", Xq), ("xkt", Xk), ("xvt", Xv))}
    in_maps = []
    for c in range(NCORES):
        b, g = divmod(c, NCORES // B)
        sl = slice(g * SC, (g + 1) * SC)
        in_maps.append({
            "xqt": np.ascontiguousarray(xt["xqt"][b][:, sl]),
            "xkt": np.ascontiguousarray(xt["xkt"][b][:, sl]),
            "xvt": np.ascontiguousarray(xt["xvt"][b][:, sl]),
            "wqt": wqt, "wkt": wkt, "wvt": wvt, "wot": wot, "bo": bo,
        })
    return in_maps


_NC_CACHE = {}


def _run(inputs, trace=False, trace_cores=None):
    from concourse.bass_utils import run_bass_kernel_spmd
    if MODE not in _NC_CACHE:
        _NC_CACHE[MODE] = build_program()
    nc = _NC_CACHE[MODE]
    in_maps = make_in_maps(inputs)
    res = run_bass_kernel_spmd(nc, in_maps, core_ids=list(range(NCORES)),
                               trace=trace, trace_cores=trace_cores)
    out = np.empty((B, S, D), dtype=np.float32)
    for c in range(NCORES):
        b, g = divmod(c, NCORES // B)
        out[b, g * SC:(g + 1) * SC, :] = res.results[c]["out"]
    return out, res


def kernel(**inputs):
    out, _ = _run(inputs, trace=False)
    return out



# revision 35
# speedup vs baseline: 1.1833x; 1.1833x over previous
"""MultiHeadAttention (faithful raw-reshape variant) on 8 trn2 NeuronCores.

Math (per batch b):  Y = Xq Wq^T, Z = Xk Wk^T, V = Xv Wv^T  [S, D]
  raw view (B,S,D)->(B,H,S,dk): head h <- rows [128h, 128h+128); within the
  block, q = 16t + j maps to (row t, features 64j..64j+64).
  A = softmax(Q K^T / 8), O = A V, out = Hcat Wo^T + b_o.
Heads partition rows -> 32 independent (b, h) tasks, 4 per core.

Per-core program (this rewrite targets the TimelineSim cost model where
matmul cost = out_free x cycles_per_row, fp8e4+DoubleRow = 0.5 cyc/row,
stationary loads are free):

  QKV projections: fp8 DoubleRow, error-compensated 3-term form
    W X ~ Wh Xh + Wl Xh + Wh Xl  (hi/lo fp8 splits prepared on host,
    pair-packed [p, kd, i, *]); 12 accumulating DR matmuls per psum tile.
  Scores: ONE stacked DR matmul per (j', j) computes all 3 compensated
    terms: stationary [Kh;Kl;Kh] (96 parts x 2-pack), moving [Qh;Qh;Ql].
    Q/K hi/lo are quantized on-device (DVE) and pair-packed via a
    DRAM bounce (SBUF->DRAM->SBUF rearrange).
  exp: ACT (direct from PSUM, fused 1/128 scale) and Pool gpsimd
    pow(e^(1/128), s) via a DVE fp16 PSUM->SBUF copy, round-robin.
  AV: transposed accumulation - stationary = exp tile [t', t], moving =
    V' [t', 65] (col 64 = ones -> softmax denominator rides along).
  normalize: DVE reciprocal + per-partition scalar mult -> fp16.
  O^T via PE transpose (identity permutation) -> hcat [f, s] -> fp16
    out-projection + bias add.
"""

import numpy as np

import concourse.bass as bass
import concourse.mybir as mybir
import concourse.tile as tile
from concourse import bacc

B, S, D = 2, 2048, 1024
H, DK = 16, 64
NCORES = 8
HPC = H // (NCORES // B)  # heads per core = 4
SC = HPC * 128            # s-rows per core = 512
P = 128
F32 = mybir.dt.float32
F16 = mybir.dt.float16
FP8 = mybir.dt.float8e4
DR = mybir.MatmulPerfMode.DoubleRow

MODE = "fp8dr"

SW = 64.0    # host weight scale before fp8 split (wq, wk, wv)
SX = 8.0     # host activation scale before fp8 split
SQ = 1.0 / 128.0           # device Q/K quantize scale (psum -> fp8)
LAM = 1.0 / 128.0          # exp scale: logit = S_psum * SQ^2*... = S_psum/128
SV = 1.0 / (SW * SX)       # V psum descale into fp16

# exp unit routing: 1 of every EXP_RR units goes to the Pool (gpsimd) path
EXP_RR = 4


def build_body(nc, out_ap, ins):
    EXPF = mybir.ActivationFunctionType.Exp
    MULT = mybir.AluOpType.mult
    ADD = mybir.AluOpType.add
    SUB = mybir.AluOpType.subtract

    with tile.TileContext(nc) as tc:
        with (
            tc.tile_pool(name="singles", bufs=1) as singles,
            tc.tile_pool(name="ap", bufs=3) as a_pool,
            tc.tile_pool(name="s16p", bufs=2) as s16_pool,
            tc.tile_pool(name="o16p", bufs=18) as o16_pool,
            tc.tile_pool(name="outp", bufs=2) as outp,
            tc.tile_pool(name="rcp", bufs=2) as rcp,
            tc.tile_pool(name="ps_sc", bufs=3, space="PSUM") as ps_sc,
            tc.tile_pool(name="ps_o", bufs=1, space="PSUM") as ps_o,
            tc.tile_pool(name="ps_op", bufs=1, space="PSUM") as ps_op,
        ):
            # ---- constants / small setup ----
            bo_sb = singles.tile([P, D], F32, tag="bo", name="bo_sb")
            bo = ins["bo"]
            bo_bcast = bass.AP(tensor=bo.tensor, offset=bo.offset,
                               ap=[[0, P], list(bo.ap[-1])])
            nc.gpsimd.dma_start(out=bo_sb, in_=bo_bcast)

            ebc = singles.tile([P, 1], F32, tag="ebc", name="ebc")
            nc.vector.memset(ebc, float(np.exp(LAM)))

            iota_p = singles.tile([P, 1], F32, tag="iop", name="iota_p")
            iota_f = singles.tile([P, P], F32, tag="iof", name="iota_f")
            nc.gpsimd.iota(iota_p, [[0, 1]], channel_multiplier=1,
                           allow_small_or_imprecise_dtypes=True)
            nc.gpsimd.iota(iota_f, [[1, P]], channel_multiplier=0,
                           allow_small_or_imprecise_dtypes=True)
            ident = singles.tile([P, P], F16, tag="id", name="ident")
            nc.vector.tensor_scalar(ident, iota_f, iota_p, None,
                                    mybir.AluOpType.is_equal)

            # V' tiles: [t', j', 64 dk + 1 ones]
            v_sb = [singles.tile([P, 16, 65], F16, tag=f"v{hl}",
                                 name=f"v_sb{hl}") for hl in range(HPC)]
            for hl in range(HPC):
                nc.vector.memset(v_sb[hl][:, :, 64:65], 1.0)

            # ---- stream inputs ----
            def load(name, shape, dtype):
                t = singles.tile(shape, dtype, tag=name, name=name)
                nc.sync.dma_start(t, ins[name])
                return t

            # K inputs first (K projection leads), first kd chunk split off
            # so the very first matmuls start ~1.5us in
            def load2(name, shape, dtype):
                t = singles.tile(shape, dtype, tag=name, name=name)
                nc.sync.dma_start(t[:, 0:1], ins[name][:, 0:1])
                nc.sync.dma_start(t[:, 1:4], ins[name][:, 1:4])
                return t

            wk2h = load2("wk2h", [P, 4, 2, D], FP8)
            xk2h = load2("xk2h", [P, 4, 2, SC], FP8)
            wk2l = load("wk2l", [P, 4, 2, D], FP8)
            xk2l = load("xk2l", [P, 4, 2, SC], FP8)
            wq2h = load("wq2h", [P, 4, 2, D], FP8)
            xq2h = load("xq2h", [P, 4, 2, SC], FP8)
            wq2l = load("wq2l", [P, 4, 2, D], FP8)
            xq2l = load("xq2l", [P, 4, 2, SC], FP8)
            # V/wot loads are emitted after q_pair(1) so they queue behind
            # the latency-critical K/Q bounce+repack DMAs (same SP queue)

            # ---- Q/K projections (fp8 DR 3-term) + quantize hi/lo ----
            qh8 = singles.tile([P, 8, SC], FP8, tag="qh8", name="qh8")
            ql8 = singles.tile([P, 8, SC], FP8, tag="ql8", name="ql8")
            kh8 = singles.tile([P, 8, SC], FP8, tag="kh8", name="kh8")
            kl8 = singles.tile([P, 8, SC], FP8, tag="kl8", name="kl8")

            def proj3(ps, region, sh, sl8, mh, ml, s_sl, m_sl):
                """12 DR matmuls: Sh Mh + Sl Mh + Sh Ml into ps[region].
                (s* = stationary hi/lo pair, m* = moving hi/lo pair)"""
                terms = ((sh, mh), (sl8, mh), (sh, ml))
                n = 0
                for st, mv in terms:
                    for kd in range(4):
                        nc.tensor.matmul(
                            ps[:, region], st[:, kd, :, s_sl], mv[:, kd, :, m_sl],
                            start=(n == 0), stop=(n == 11), perf_mode=DR)
                        n += 1

            # scores operand tiles [sec*32+p, j, i, s]: value = T[64j+32i+p, s]
            # (d = p + 32i pairing keeps every repack DMA 3-dim-balanceable)
            q2 = singles.tile([96, 16, 2, SC], FP8, tag="q2", name="q2")
            k2 = singles.tile([96, 16, 2, SC], FP8, tag="k2", name="k2")
            scr = {n: ins[n] for n in ("qscrh", "qscrl", "kscrh", "kscrl")}
            COPYF = mybir.ActivationFunctionType.Copy

            def proj_mf(wh, wl, xh, xl, h8, l8, mf):
                """One 128-col chunk of a projection + hi/lo quantize.
                hi rides the (lead-idle) ACT engine, lo residual on DVE."""
                ps = ps_sc.tile([P, 1024], F32, tag="sc", name="ps_p")
                for half in range(2):
                    proj3(ps, slice(256 * half, 256 * half + 256),
                          wh, wl, xh, xl,
                          slice(128 * mf, 128 * mf + 128),
                          slice(256 * half, 256 * half + 256))
                nc.scalar.activation(h8[:, mf, :], ps[:, 0:512], COPYF,
                                     scale=SQ)
                nc.vector.scalar_tensor_tensor(
                    l8[:, mf, :], ps[:, 0:512], SQ, h8[:, mf, :], MULT, SUB)

            def repack(dst, hi_name, lo_name, order, rows, j_sl):
                """Pair-pack scratch rows f=64j+32i+p into [sec, j, i, s].
                Same queue as the bounce DMAs => ordered on HW."""
                names = [hi_name, lo_name]
                for sec in range(3):
                    src = scr[names[order[sec]]][rows].rearrange(
                        "(j i p) s -> p j i s", p=32, i=2)
                    nc.sync.dma_start(dst[32 * sec:32 * sec + 32, j_sl],
                                      src)

            # K projection first (scores need all of k2, only q2[j] of Q)
            for mf in range(8):
                proj_mf(wk2h, wk2l, xk2h, xk2l, kh8, kl8, mf)
            for name, t8 in (("kscrh", kh8), ("kscrl", kl8)):
                nc.sync.dma_start(
                    scr[name].rearrange("(mf p) s -> p mf s", p=P), t8)
            repack(k2, "kscrh", "kscrl", (0, 1, 0), slice(0, D),
                   slice(0, 16))

            def q_pair(g):
                """Q chunks 2g, 2g+1 -> bounce -> repack of q2 j 4g..4g+4."""
                proj_mf(wq2h, wq2l, xq2h, xq2l, qh8, ql8, 2 * g)
                proj_mf(wq2h, wq2l, xq2h, xq2l, qh8, ql8, 2 * g + 1)
                rows = slice(256 * g, 256 * g + 256)
                for name, t8 in (("qscrh", qh8), ("qscrl", ql8)):
                    nc.sync.dma_start(
                        scr[name][rows].rearrange("(z p) s -> p z s", p=P),
                        t8[:, 2 * g:2 * g + 2, :])
                repack(q2, "qscrh", "qscrl", (0, 0, 1), rows,
                       slice(4 * g, 4 * g + 4))

            wv2h = load("wv2h", [P, 4, 2, D], FP8)
            wv2l = load("wv2l", [P, 4, 2, D], FP8)
            xv2h = load("xv2h", [P, 4, 2, SC], FP8)
            xv2l = load("xv2l", [P, 4, 2, SC], FP8)
            wot = load("wot", [P, 8, D], F16)
            q_pair(0)
            q_pair(1)

            def v_chunk(hl, fq):
                # stationary = X chunk (s-block -> out partitions), moving = W
                ps = ps_sc.tile([P, 1024], F32, tag="sc", name="ps_v")
                proj3(ps, slice(0, 256), xv2h, xv2l, wv2h, wv2l,
                      slice(128 * hl, 128 * hl + 128),
                      slice(256 * fq, 256 * fq + 256))
                nc.scalar.activation(
                    v_sb[hl][:, 4 * fq:4 * fq + 4, 0:64],
                    ps[:, 0:256].rearrange("p (j d) -> p j d", d=64),
                    COPYF, scale=SV)

            v_todo = [(hl, fq) for hl in range(HPC) for fq in range(4)]

            hcat = singles.tile([P, 8, SC], F16, tag="hcat", name="hcat")

            # ---- main attention loop (software-pipelined emission) ----
            # PE order per j: scores(j) then AV(j-1) - so PE computes scores
            # while exp(j-1) finishes on ACT/Pool/DVE. Transposes batch at
            # head end; out-proj(head) is emitted after the next head's first
            # scores so PE never parks on the DVE normalize chain.
            unit = 0
            pending_av = None    # (hl, a_t)
            pending_ops = []     # deferred emitters

            def emit_scores(hl, j, hs):
                nonlocal unit
                a_t = a_pool.tile([P, 16, P], F16, tag="a", name="a_t")
                for h2 in range(2):
                    ps = ps_sc.tile([P, 1024], F32, tag="sc", name="ps_s")
                    psv = ps.rearrange("p (js t) -> p js t", t=P)
                    for js in range(8):
                        jp = 8 * h2 + js
                        nc.tensor.matmul(
                            psv[:, js, :], k2[:, jp, :, hs],
                            q2[:, j, :, hs], start=True, stop=True,
                            perf_mode=DR)
                    dst = a_t[:, 8 * h2:8 * h2 + 8, :]
                    if unit % 5 in (2, 4):
                        s16 = s16_pool.tile([P, 1024], F16, tag="s16",
                                            name="s16")
                        nc.vector.tensor_copy(s16, ps)
                        eb = bass.AP(tensor=ebc.tensor, offset=ebc.offset,
                                     ap=[list(ebc.ap[0]), [0, 8], [0, P]])
                        nc.gpsimd.tensor_tensor(
                            dst, eb,
                            s16.rearrange("p (js t) -> p js t", t=P),
                            mybir.AluOpType.pow)
                    else:
                        nc.scalar.activation(dst, psv, EXPF, scale=LAM)
                    unit += 1
                return a_t

            # both AV accumulators packed into one psum bank, hand-rotated
            o_big = ps_o.tile([P, 130], F32, tag="o", name="o_big")
            o_par = [0]

            def emit_av(hl, a_t, o16s):
                par = o_par[0]
                o_par[0] ^= 1
                o_ps = o_big[:, 65 * par:65 * par + 65]
                for jp in range(16):
                    nc.tensor.matmul(o_ps, a_t[:, jp, :], v_sb[hl][:, jp, :],
                                     start=(jp == 0), stop=(jp == 15))
                return o_ps

            norm_alt = [0]

            def emit_norm(o_ps, o16s):
                rc = rcp.tile([P, 1], F32, tag="rc", name="rc")
                nc.vector.reciprocal(rc, o_ps[:, 64:65])
                o16 = o16_pool.tile([P, 64], F16, tag="o16", name="o16")
                # alternate the normalize between ACT (scale-AP) and DVE
                norm_alt[0] ^= 1
                if norm_alt[0]:
                    nc.scalar.activation(o16, o_ps[:, 0:64],
                                         mybir.ActivationFunctionType.Copy,
                                         scale=rc)
                else:
                    nc.vector.tensor_scalar(o16, o_ps[:, 0:64], rc, None,
                                            MULT)
                o16s.append(o16)

            def head_tail(hl, hs, o16s):
                """Batched transposes into hcat, then out-projection."""
                def tail():
                    # 8 transposes per j-parity into the out-proj bank
                    # (f16 bitcast view), then one big DVE copy per parity
                    for a2 in range(2):
                        trt = ps_op.tile([P, 512], F32, tag="op", name="trt")
                        tr = trt[0:64, :].bitcast(F16).rearrange(
                            "p (j t) -> p j t", t=P)
                        for po in range(8):
                            nc.tensor.matmul(tr[:, po, :], o16s[2 * po + a2],
                                             ident, start=True, stop=True,
                                             is_transpose=True)
                        nc.vector.tensor_copy(
                            hcat[64 * a2:64 * a2 + 64, :, hs], tr)
                    for fh in range(2):
                        fs = slice(512 * fh, 512 * fh + 512)
                        ps = ps_op.tile([P, 512], F32, tag="op", name="psop")
                        for po in range(8):
                            nc.tensor.matmul(ps, hcat[:, po, hs],
                                             wot[:, po, fs],
                                             start=(po == 0), stop=(po == 7))
                        os_t = outp.tile([P, 512], F32, tag="os", name="os_t")
                        nc.vector.tensor_tensor(os_t, ps, bo_sb[:, fs], ADD)
                        nc.sync.dma_start(out_ap[hs, fs], os_t)
                return tail

            # 3-stage pipeline: step j emits scores(j), AV(j-1), norm(j-2) -
            # each cross-engine hop gets a full step of slack
            pending_norm = None  # (o_ps, o16s)
            for hl in range(HPC):
                hs = slice(128 * hl, 128 * hl + 128)
                o16s = []
                for j in range(16):
                    if hl == 0:
                        # stream remaining work into the young pipeline:
                        # 2 V-proj chunks and (every 4th j) the next Q chunk
                        for _ in range(2):
                            if v_todo:
                                v_chunk(*v_todo.pop(0))
                        if j % 4 == 0 and j // 4 + 2 < 4:
                            q_pair(j // 4 + 2)
                    a_t = emit_scores(hl, j, hs)
                    if pending_av is not None:
                        phl, pa_t, po16s = pending_av
                        o_ps = emit_av(phl, pa_t, po16s)
                        if pending_norm is not None:
                            emit_norm(*pending_norm)
                        pending_norm = (o_ps, po16s)
                    pending_av = (hl, a_t, o16s)
                    # head tails fire only once their 16 norms are emitted
                    while pending_ops and len(pending_ops[0][1]) == 16:
                        pending_ops.pop(0)[0]()
                pending_ops.append((head_tail(hl, hs, o16s), o16s))
            phl, pa_t, po16s = pending_av
            o_ps = emit_av(phl, pa_t, po16s)
            if pending_norm is not None:
                emit_norm(*pending_norm)
            emit_norm(o_ps, po16s)
            for emit, _ in pending_ops:
                emit()
    return nc


def build_program():
    nc = bacc.Bacc("TRN2", target_bir_lowering=False, debug=False,
                   enable_asserts=False, num_devices=NCORES)
    ins = {}
    for nm in ("wq2h", "wq2l", "wk2h", "wk2l", "wv2h", "wv2l"):
        ins[nm] = nc.dram_tensor(nm, [P, 4, 2, D], FP8,
                                 kind="ExternalInput").ap()
    for nm in ("xq2h", "xq2l", "xk2h", "xk2l", "xv2h", "xv2l"):
        ins[nm] = nc.dram_tensor(nm, [P, 4, 2, SC], FP8,
                                 kind="ExternalInput").ap()
    ins["wot"] = nc.dram_tensor("wot", [P, 8, D], F16,
                                kind="ExternalInput").ap()
    ins["bo"] = nc.dram_tensor("bo", [1, D], F32, kind="ExternalInput").ap()
    for nm in ("qscrh", "qscrl", "kscrh", "kscrl"):
        ins[nm] = nc.dram_tensor(nm, [D, SC], FP8, kind="Internal").ap()
    out_ap = nc.dram_tensor("out", [SC, D], F32, kind="ExternalOutput").ap()
    build_body(nc, out_ap, ins)
    nc.finalize()
    return nc


def _split8(a):
    """fp32 array -> (hi, lo) fp8e4 with lo = a - hi."""
    import ml_dtypes
    hi = a.astype(ml_dtypes.float8_e4m3)
    lo = (a - hi.astype(np.float32)).astype(ml_dtypes.float8_e4m3)
    return hi, lo


def _pack_pairs(a, kd_chunks):
    """[d, n] -> [128, kd, 2, n] with d = 256*kd + 2*p + i."""
    d, n = a.shape
    return np.ascontiguousarray(
        a.reshape(kd_chunks, P, 2, n).transpose(1, 0, 2, 3))


def make_in_maps(inputs):
    Xs = {n: np.asarray(inputs[n], dtype=np.float32)
          for n in ("X_q", "X_k", "X_v")}
    Ws = {n: np.asarray(inputs[n], np.float32).T  # [d_in, f_out]
          for n in ("W_q", "W_k", "W_v")}
    w_packed = {}
    for nm, wt in Ws.items():
        hi, lo = _split8(SW * wt)
        w_packed[nm] = (_pack_pairs(hi.astype(np.float32), 4).astype(hi.dtype),
                        _pack_pairs(lo.astype(np.float32), 4).astype(lo.dtype))
    wot = np.ascontiguousarray(
        np.asarray(inputs["W_o"], np.float32).T.reshape(8, P, D)
        .transpose(1, 0, 2)).astype(np.float16)
    bo = np.asarray(inputs["b_o"], np.float32).reshape(1, D)

    in_maps = []
    for c in range(NCORES):
        b, g = divmod(c, NCORES // B)
        sl = slice(g * SC, (g + 1) * SC)
        m = {
            "wq2h": w_packed["W_q"][0], "wq2l": w_packed["W_q"][1],
            "wk2h": w_packed["W_k"][0], "wk2l": w_packed["W_k"][1],
            "wv2h": w_packed["W_v"][0], "wv2l": w_packed["W_v"][1],
            "wot": wot, "bo": bo,
        }
        for inm, xnm in (("xq2", "X_q"), ("xk2", "X_k"), ("xv2", "X_v")):
            xt = np.ascontiguousarray(Xs[xnm][b].T[:, sl])  # [1024, 512]
            hi, lo = _split8(SX * xt)
            m[inm + "h"] = _pack_pairs(hi.astype(np.float32), 4).astype(hi.dtype)
            m[inm + "l"] = _pack_pairs(lo.astype(np.float32), 4).astype(lo.dtype)
        in_maps.append(m)
    return in_maps


_NC_CACHE = {}


def _run(inputs, trace=False, trace_cores=None):
    from concourse.bass_utils import run_bass_kernel_spmd
    if MODE not in _NC_CACHE:
        _NC_CACHE[MODE] = build_program()
    nc = _NC_CACHE[MODE]
    in_maps = make_in_maps(inputs)
    res = run_bass_kernel_spmd(nc, in_maps, core_ids=list(range(NCORES)),
                               trace=trace, trace_cores=trace_cores)
    out = np.empty((B, S, D), dtype=np.float32)
    for c in range(NCORES):
        b, g = divmod(c, NCORES // B)
        out[b, g * SC:(g + 1) * SC, :] = res.results[c]["out"]
    return out, res


def kernel(**inputs):
    out, _ = _run(inputs, trace=False)
    return out
